# revision 1
# baseline (speedup 1.0000x reference)
"""EnvelopeDetector Trainium2 kernel (Bass/Tile), channel-sharded over 8
NeuronCores. Each core owns 8 of the 64 channels, so the BatchNorm batch
stats (per-channel over N,L) are fully local -- no collectives.

Per-channel dataflow (5-stage software pipeline across channels):
  load : one contiguous DMA of host-staged bf16 x in the (j,b)-partition
         transpose layout: staged[32j+b, 128g+u] = x[b, 512g+128j+u].
  txs  : PE transposes (bf16, 4 per PSUM bank) -> x_T[t(part), 32*chunk+b].
  front: conv1 (depthwise K=100) as PE matmuls with host-built 128x128
         Toeplitz band stationaries A1/B1 (bf16), moving = x_T slices
         (N=512, fp32 PSUM accumulation, 2 matmuls per 16-chunk bank);
         y evacuated to bf16 with a fused per-partition sum accumulation
         (DVE tensor_scalar accum_out), per-segment sum-of-squares on ACT
         (Square + accum_out). Out-of-range tail handled by exact-region
         partial accumulations.
  mid  : ones-vector matmul reduces stats across partitions; tiny scalar
         chain -> scale = gamma/std and b' = (beta/gamma)*std - mean
         (uses |s*y + bias| = s*|y + b'|, s > 0); PE-broadcast to [128,1];
         a' = |y + b'| in two wide ACT Abs ops -> bf16 a_T.
  back : conv2 (K=50): four a_T chunks form one 128-col stationary, moving
         = Toeplitz A2/B2 (bf16); a 4-col bank-marking matmul gives clean
         overwrite-then-accumulate PSUM semantics and orders each bank.
         Output lands in natural [b,t] layout; the evacuation applies
         z = s*psum + b_low; staged [128, 2560] and stored with one
         strided DMA per row-group (HWDGE for the first half, gpsimd/SWDGE
         for the second, keeping the in-order SP queue free for x loads).
"""

import sys

import numpy as np

try:
    import concourse.bass as bass  # noqa: F401
except ImportError:  # pragma: no cover
    sys.path.insert(0, "/opt/trn_rl_repo")

B, C, T = 32, 64, 20000
K1, K2 = 100, 50
T1 = T - K1 + 1  # 19901
T2 = T1 - K2 + 1  # 19852
NCORES = 8
CL = C // NCORES  # 8 channels per core
BN_EPS = 1e-5

P = 128
NQ1 = 10  # conv1 psum bank groups (16 chunks x 32 batch cols = 512)
NCH_Z = 156  # z chunks 0..155 (chunk 155 has 12 valid cols)
XT_COLS = 161 * 32  # 5152
YT_COLS = 160 * 32  # 5120
X4_COLS = 40 * P  # 5120 (40 g-blocks of 512 t)

_CACHE = {}


def _build_program(repeats=1):
    import concourse.bass as bass  # noqa: F401
    import concourse.tile as tile
    from concourse import bacc, mybir
    from contextlib import ExitStack

    f32 = mybir.dt.float32
    AFT = mybir.ActivationFunctionType
    ALU = mybir.AluOpType
    AX = mybir.AxisListType

    bf16 = mybir.dt.bfloat16

    nc = bacc.Bacc("TRN2", target_bir_lowering=False, debug=False,
                   num_devices=NCORES)

    x_d = nc.dram_tensor("x_loc", [CL, P, X4_COLS], bf16,
                         kind="ExternalInput").ap()
    tp_d = nc.dram_tensor("toep", [CL, 2, P, P], bf16,
                          kind="ExternalInput").ap()
    tp2_d = nc.dram_tensor("toep2", [CL, 2, P, P], bf16,
                           kind="ExternalInput").ap()
    cb_d = nc.dram_tensor("cb", [4, CL], f32, kind="ExternalInput").ap()
    id_d = nc.dram_tensor("ident", [P, P], bf16, kind="ExternalInput").ap()
    on_d = nc.dram_tensor("ones", [P, P], f32, kind="ExternalInput").ap()
    z_d = nc.dram_tensor("z_loc", [B, CL, T2], f32, kind="ExternalOutput").ap()

    with tile.TileContext(nc) as tc:
        with ExitStack() as ctx:
            p_const = ctx.enter_context(tc.tile_pool(name="const", bufs=1))
            p_x4 = ctx.enter_context(tc.tile_pool(name="x4", bufs=3))
            p_xt = ctx.enter_context(tc.tile_pool(name="xt", bufs=2))
            p_yt = ctx.enter_context(tc.tile_pool(name="yt", bufs=2))
            p_at = ctx.enter_context(tc.tile_pool(name="at", bufs=2))
            p_zt = ctx.enter_context(tc.tile_pool(name="zt", bufs=2))
            p_st = ctx.enter_context(tc.tile_pool(name="st", bufs=2))
            p_sq = ctx.enter_context(tc.tile_pool(name="sq", bufs=2))
            pp_y = ctx.enter_context(tc.tile_pool(name="ppy", bufs=3, space="PSUM"))
            pp_tx = ctx.enter_context(tc.tile_pool(name="pptx", bufs=2, space="PSUM"))
            pp_z = ctx.enter_context(tc.tile_pool(name="ppz", bufs=2, space="PSUM"))
            pp_m = ctx.enter_context(tc.tile_pool(name="ppm", bufs=1, space="PSUM"))

            # ---- constants ----
            toep_sb = p_const.tile([P, CL * 2 * P], bf16, tag="toep")
            nc.sync.dma_start(
                toep_sb[:].rearrange("p (c k f) -> p c k f", c=CL, k=2, f=P),
                tp_d.rearrange("c k p f -> p c k f"),
            )
            toep2_sb = p_const.tile([P, CL * 2 * P], bf16, tag="toep2")
            nc.sync.dma_start(
                toep2_sb[:].rearrange("p (c k f) -> p c k f", c=CL, k=2, f=P),
                tp2_d.rearrange("c k p f -> p c k f"),
            )
            id_sb = p_const.tile([P, P], bf16, tag="ident")
            nc.sync.dma_start(id_sb[:], id_d)
            on_sb = p_const.tile([P, P], f32, tag="ones")
            nc.sync.dma_start(on_sb[:], on_d)
            cb_sb = p_const.tile([1, 4 * CL], f32, tag="cb")
            nc.sync.dma_start(cb_sb[:], cb_d.flatten().unsqueeze(0))
            z0 = p_const.tile([P, 512], bf16, tag="zeros")
            nc.vector.memset(z0[:], 0.0)
            # broadcast b_low for all channels once: [128, CL]
            pmb = pp_m.tile([P, 32], f32, tag="m")
            nc.tensor.matmul(pmb[:, 0:CL], on_sb[0:1, :],
                             cb_sb[0:1, 2 * CL:3 * CL])
            blow_bc = p_const.tile([P, CL], f32, tag="blow")
            nc.vector.tensor_copy(blow_bc[:], pmb[:, 0:CL])
            eps_sb = p_const.tile([1, 1], f32, tag="eps")
            nc.vector.memset(eps_sb[:], BN_EPS)

            NTOT = float(B * T1)

            def load(c):
                """prefetch host-staged x for channel c (one contiguous DMA).
                x_loc[c, 32j+b, 128g+u] = x[b, c, 512g+128j+u], zero-padded
                past t=20000."""
                t4 = p_x4.tile([P, X4_COLS], bf16, tag="x4")
                nc.sync.dma_start(t4[:], x_d[c])
                return t4

            def txs(c, t4):
                """PE transposes for channel c."""
                # ---- PE transposes -> x_T [t(part), 32*chunk + b] ----
                xt = p_xt.tile([P, XT_COLS], bf16, tag="xt")
                nc.vector.memset(xt[:, 5120:5152], 0.0)  # chunk 160
                for gg in range(10):
                    ptx = pp_tx.tile([P, 512], bf16, tag="tx")
                    for r in range(4):
                        g = 4 * gg + r
                        nc.tensor.transpose(ptx[:, 128 * r:128 * (r + 1)],
                                            t4[:, 128 * g:128 * g + 128],
                                            id_sb[:])
                    nc.vector.tensor_copy(
                        xt[:, 512 * gg:512 * (gg + 1)], ptx[:])
                return xt

            def front(c, xt):
                """conv1 + BN stats accumulation for channel c."""
                A1 = toep_sb[:, (2 * c + 0) * P:(2 * c + 1) * P]
                B1 = toep_sb[:, (2 * c + 1) * P:(2 * c + 2) * P]
                # ---- conv1 + stats accumulation ----
                # statcols: sums in 0..10 (9=q9-main, 10=q9-partial rows<61),
                #           sumsq in 11..21 (20=q9-main, 21=q9-partial)
                yt = p_yt.tile([P, YT_COLS], bf16, tag="yt")
                statcols = p_st.tile([P, 16], f32, tag="statcols")
                nc.vector.memset(statcols[:], 0.0)
                for si, seg in enumerate(((0, 1, 2), (3, 4, 5),
                                          (6, 7, 8), (9,))):
                    psums = {}
                    for q in seg:
                        py = pp_y.tile([P, 512], f32, tag="y")
                        psums[q] = py
                        nc.tensor.matmul(py[:], A1,
                                         xt[:, 512 * q:512 * q + 512],
                                         start=True, stop=False)
                    for q in seg:
                        nc.tensor.matmul(psums[q][:], B1,
                                         xt[:, 512 * q + 32:512 * q + 544],
                                         start=False, stop=True)
                    for q in seg:
                        py = psums[q]
                        if q < 9:
                            nc.vector.tensor_scalar(
                                yt[:, 512 * q:512 * q + 512], py[:], 0.0, 0.0,
                                op0=ALU.add, op1=ALU.add,
                                accum_out=statcols[:, q:q + 1])
                        else:
                            # valid y: chunks 144..154 (cols<352) full, plus
                            # chunk 155 rows<61 (cols 352:384)
                            nc.vector.tensor_scalar(
                                yt[:, 4608:4960], py[:, 0:352], 0.0, 0.0,
                                op0=ALU.add, op1=ALU.add,
                                accum_out=statcols[:, 9:10])
                            nc.vector.tensor_copy(yt[:, 4960:5120],
                                                  py[:, 352:512])
                            # partial sum for chunk 155 rows<61; out goes to
                            # the dead chunk-156 region of yt
                            nc.vector.tensor_scalar(
                                yt[0:61, 4992:5024], py[0:61, 352:384],
                                0.0, 0.0, op0=ALU.add, op1=ALU.add,
                                accum_out=statcols[0:61, 10:11])
                    # per-segment sumsq from bf16 y (one wide ACT op)
                    sq = p_sq.tile([P, 1536], f32, tag="sq")
                    if si < 3:
                        nc.scalar.activation(
                            sq[:], yt[:, 1536 * si:1536 * (si + 1)],
                            AFT.Square, accum_out=statcols[:, 11 + si:12 + si])
                    else:
                        nc.scalar.activation(
                            sq[:, 0:352], yt[:, 4608:4960], AFT.Square,
                            accum_out=statcols[:, 14:15])
                        nc.scalar.activation(
                            sq[0:61, 352:384], yt[0:61, 4960:4992],
                            AFT.Square, accum_out=statcols[0:61, 15:16])

                return {"yt": yt, "statcols": statcols}

            def mid(c, stt):
                """BN stats scalar chain + |scale*y + bias| for channel c."""
                yt, statcols = stt["yt"], stt["statcols"]
                at = p_at.tile([P, YT_COLS], bf16, tag="at")
                pm = pp_m.tile([P, 32], f32, tag="m")
                nc.tensor.matmul(pm[0:1, 0:16], on_sb[:, 0:1], statcols[:])
                ss = p_st.tile([1, 2], f32, tag="ss")
                nc.vector.reduce_sum(ss[:, 0:1], pm[0:1, 0:11], axis=AX.X)
                nc.vector.reduce_sum(ss[:, 1:2], pm[0:1, 11:16], axis=AX.X)
                mE = p_st.tile([1, 2], f32, tag="mE")
                nc.vector.tensor_scalar_mul(mE[:], ss[:], 1.0 / NTOT)
                msq = p_st.tile([1, 1], f32, tag="msq")
                nc.vector.tensor_mul(msq[:], mE[:, 0:1], mE[:, 0:1])
                var = p_st.tile([1, 1], f32, tag="var")
                nc.vector.tensor_sub(var[:], mE[:, 1:2], msq[:])
                s0 = p_st.tile([1, 1], f32, tag="s0")
                nc.scalar.activation(s0[:], var[:], AFT.Sqrt, bias=eps_sb[:])
                inv = p_st.tile([1, 1], f32, tag="inv")
                nc.vector.reciprocal(inv[:], s0[:])
                # sb3: [scale = gamma/std, b' = (beta/gamma)*std - mean]
                # using |s*y + bias| = s*|y + b'|  (s > 0), s folded into the
                # z evacuation.
                sb3 = p_st.tile([1, 2], f32, tag="sb3")
                nc.vector.tensor_mul(sb3[:, 0:1], inv[:], cb_sb[:, c:c + 1])
                nc.vector.scalar_tensor_tensor(
                    sb3[:, 1:2], s0[:], cb_sb[:, 3 * CL + c:3 * CL + c + 1],
                    mE[:, 0:1], op0=ALU.mult, op1=ALU.subtract)
                nc.tensor.matmul(pm[:, 22:24], on_sb[0:1, :], sb3[:])
                bc = p_st.tile([P, 2], f32, tag="bcast")
                nc.vector.tensor_copy(bc[:], pm[:, 22:24])

                # ---- a' = |y + b'| -> bf16 a_T for conv2 ----
                for h in range(2):
                    nc.scalar.activation(at[:, 2560 * h:2560 * (h + 1)],
                                         yt[:, 2560 * h:2560 * (h + 1)],
                                         AFT.Abs, bias=bc[:, 1:2])
                return {"at": at, "bc": bc}

            def back(c, stt):
                """conv2 + scale + b_low bias + store for channel c."""
                at, bc = stt["at"], stt["bc"]
                A2 = toep2_sb[:, (2 * c + 0) * P:(2 * c + 1) * P]
                B2 = toep2_sb[:, (2 * c + 1) * P:(2 * c + 2) * P]
                zc = z_d[:, c, :]
                blv = blow_bc[:, c:c + 1]

                # ---- conv2: 4 a_T chunks as one 128-col stationary ----
                # psum[32j+b, u] = sum_v a_T[v, 32(m+j)+b] * A2[v, u]  (+ B2
                # with the window shifted one chunk) = z chunk m+j.
                # z staged per 5-bank group in zt [128, 2560]; one gpsimd
                # (SWDGE) DMA per jz row-group.
                for G in range(2):
                    q2lo, q2hi = 5 * G, 5 * G + 5
                    zt = p_zt.tile([P, 2560], f32, tag="zt")
                    for q2 in range(q2lo, q2hi):
                        g4lo = 4 * q2
                        g4hi = min(g4lo + 4, 39)
                        pz = pp_z.tile([P, 512], f32, tag="z")
                        # bank-marking matmul: one col per region; orders the
                        # bank and gives clean overwrite-then-accumulate
                        nc.tensor.matmul(
                            pz[:].rearrange("p (s u) -> p s u",
                                            s=4, u=128)[:, :, 0:1],
                            z0[:, 0:P], z0[:, 0:4], start=True, stop=False,
                            skip_group_check=True)
                        for g4 in range(g4lo, g4hi):
                            m = 4 * g4
                            s = g4 % 4
                            out_ap = pz[:, 128 * s:128 * s + 128]
                            last = (g4 == g4hi - 1)
                            nc.tensor.matmul(out_ap,
                                             at[:, 32 * m:32 * m + 128], A2,
                                             start=False, stop=False,
                                             skip_group_check=True)
                            nc.tensor.matmul(
                                out_ap, at[:, 32 * (m + 1):32 * (m + 1) + 128],
                                B2, start=False, stop=last,
                                skip_group_check=True)
                        ncols = 512 if q2 < 9 else 384
                        off = 512 * (q2 % 5)
                        if q2 in (0, 2, 6, 8):
                            nc.vector.tensor_scalar(
                                zt[:, off:off + ncols], pz[:, 0:ncols],
                                bc[:, 0:1], blv, op0=ALU.mult, op1=ALU.add)
                        else:
                            nc.scalar.activation(
                                zt[:, off:off + ncols], pz[:, 0:ncols],
                                AFT.Identity, bias=blv, scale=bc[:, 0:1])
                    # store group G: chunks [80G, 80G+80) except tail
                    if G == 0:
                        # z[b, 512s' + 128jz + u] <- zt[32jz+b, 128s'+u]
                        zg = zc[:, 0:10240].rearrange(
                            "b (s r) -> b s r", s=20, r=512)
                        for jz in range(4):
                            nc.sync.dma_start(
                                zg[:, :, 128 * jz:128 * jz + 128],
                                zt[32 * jz:32 * jz + 32, :].rearrange(
                                    "b (s u) -> b s u", s=20, u=P),
                            )
                    else:
                        # chunks 80..151: 18 full s' blocks per jz
                        zg = zc[:, 10240:19456].rearrange(
                            "b (s r) -> b s r", s=18, r=512)
                        for jz in range(4):
                            nc.gpsimd.dma_start(
                                zg[:, :, 128 * jz:128 * jz + 128],
                                zt[32 * jz:32 * jz + 32, 0:2304].rearrange(
                                    "b (s u) -> b s u", s=18, u=P),
                            )
                        # chunks 152..155 (s'=18), chunk 155 partial (12)
                        for m in range(152, NCH_Z):
                            jz = m % 4
                            w = P if m < NCH_Z - 1 else T2 - P * (NCH_Z - 1)
                            nc.gpsimd.dma_start(
                                zc[:, P * m:P * m + w],
                                zt[32 * jz:32 * jz + 32, 2304:2304 + w])

            # 4-stage software pipeline: load(c) / transpose+conv1+stats(c-1)
            # / stats-chain+abs(c-2) / conv2+store(c-3).
            NCH = CL * repeats
            lds, txd, frs, mds = {}, {}, {}, {}
            for c in range(NCH + 4):
                if c < NCH:
                    lds[c] = load(c % CL)
                if c >= 4:
                    back((c - 4) % CL, mds.pop(c - 4))
                if 3 <= c <= NCH + 2:
                    mds[c - 3] = mid((c - 3) % CL, frs.pop(c - 3))
                if 2 <= c <= NCH + 1:
                    frs[c - 2] = front((c - 2) % CL, txd.pop(c - 2))
                if 1 <= c <= NCH:
                    txd[c - 1] = txs((c - 1) % CL, lds.pop(c - 1))

    nc.compile()
    return nc


def _host_prep(x, w_band, gamma, beta, w_low, b_low):
    """Build per-core input maps (Toeplitz band matrices built on host)."""
    x = np.asarray(x, dtype=np.float32)
    wb = np.asarray(w_band, dtype=np.float32).reshape(C, K1)
    wl = np.asarray(w_low, dtype=np.float32).reshape(C, K2)
    gamma = np.asarray(gamma, dtype=np.float32).reshape(C)
    beta = np.asarray(beta, dtype=np.float32).reshape(C)
    b_low = np.asarray(b_low, dtype=np.float32).reshape(C)

    v = np.arange(P)[:, None]
    m = np.arange(P)[None, :]

    def toep_pair(w, K):
        dA = v - m
        dB = v + P - m
        A = np.where((dA >= 0) & (dA < K), w[:, np.clip(dA, 0, K - 1)], 0.0)
        Bm = np.where((dB >= 0) & (dB < K), w[:, np.clip(dB, 0, K - 1)], 0.0)
        return A.astype(np.float32), Bm.astype(np.float32)

    A1, B1 = toep_pair(wb, K1)
    A2, B2 = toep_pair(wl, K2)
    import ml_dtypes
    bf16 = ml_dtypes.bfloat16
    ident = np.eye(P, dtype=bf16)
    ones = np.ones((P, P), dtype=np.float32)
    xb = x.astype(bf16)

    # stage x into the on-chip transpose layout:
    # staged[c, 32j+b, 128g+u] = x[b, c, 512g+128j+u]  (zero-pad past 20000)
    staged = np.zeros((C, P, 40 * P), dtype=bf16)
    xm = xb[:, :, :19968].reshape(B, C, 39, 4, P)
    staged.reshape(C, 4, 32, 40, P)[:, :, :, :39, :] = (
        xm.transpose(1, 3, 0, 2, 4))
    staged.reshape(C, 4, 32, 40, P)[:, 0, :, 39, :32] = (
        xb[:, :, 19968:20000].transpose(1, 0, 2))

    in_maps = []
    for i in range(NCORES):
        ch = slice(CL * i, CL * (i + 1))
        in_maps.append({
            "x_loc": np.ascontiguousarray(staged[ch]),
            "toep": np.ascontiguousarray(
                np.stack([A1[ch], B1[ch]], axis=1)).astype(bf16),
            "toep2": np.ascontiguousarray(
                np.stack([A2[ch], B2[ch]], axis=1)).astype(bf16),
            "cb": np.ascontiguousarray(
                np.stack([gamma[ch], beta[ch], b_low[ch],
                          beta[ch] / np.where(gamma[ch] != 0.0,
                                              gamma[ch], 1.0)])),
            "ident": ident,
            "ones": ones,
        })
    return in_maps


def run(inputs, trace=False):
    """Run on 8 NeuronCores; returns (z_full, exec_time_ns_or_None)."""
    from concourse.bass_utils import run_bass_kernel_spmd

    if "nc" not in _CACHE:
        _CACHE["nc"] = _build_program()
    nc = _CACHE["nc"]
    in_maps = _host_prep(**inputs)
    res = run_bass_kernel_spmd(nc, in_maps, list(range(NCORES)), trace=trace)
    z = np.concatenate([np.asarray(r["z_loc"]) for r in res.results], axis=1)
    return z.astype(np.float32), res.exec_time_ns


def kernel(**inputs):
    z, _ = run(inputs)
    return z

